# revision 19
# baseline (speedup 1.0000x reference)
"""NodeVarGraphConvolutionLayer on 8 TRN2 NeuronCores.

Math (see reference):
  Xs = X.sum(-1)                        [B, N]
  P0 = Xs;  P_i = A @ P_{i-1}           (3 batched matvecs, N=1024)
  Y[b,n,c] = sum_i h[i,c,n] * P_i[b,n]  [B, N, 64]
  out = tanh(LayerNorm_c(Y))            (gamma=1, beta=0 folded away)

Sharding: data-parallel over batch. B=16 -> 2 batches per core.

v4 design (fp32 baseline ~96 us, v2/v3 ~61 us):
  * fp16 matvec chain on A/32 (host-scaled): PE streams A at 1 col/cycle
    (4x the fp32 rate warm) and HBM traffic halves to ~4.9 MB/core.
    P_i' = P_i/32^i stays in fp16 range; fp16's absolute chain error is
    ~8x below bf16, which matters where |P_3| is small (LN+tanh flips).
  * matvec: p' (fp16 col, 4B-aligned 2-elem padding) stationary, A^T
    chunks moving, psum rows at partitions 0/32; rows cast to fp16 on
    ACT, PE-transposed back to cols; the cole copy un-scales by 32^i
    into bf16 for the epilogue.
  * LN stats never touch Y: host moments HM[i,n]=mean_c h, M2[i,j,n]=
    mean_c h_i h_j give mean/E[Y^2] from the tiny col tensor. eps is
    dropped (var ~ 1e4..1e12 here, eps=1e-5 is far below fp32 ulp).
    rstd = Quake seed + 1 Newton iteration (0.17% worst-case, well
    inside the error budget).
  * Y accumulates incrementally on DVE as each P_i lands (h_i * c_i
    mult + add per step), so only the i=3 term (on GPSIMD) remains
    after the last transpose, in parallel with the DVE stats chain.
  * DMA: every dma_start costs ~0.7 us of ring issue time and Tile has
    8 completion lanes, so the kernel uses 9 dma_starts total: packed
    blobX (X + EYE, SP ring first), HB / BF blobs (ACT), one 512KB A
    chunk per batch on SP + three on SWDGE (batch 0 first), OUT halves
    on SP at the tail.
"""

import numpy as np

B, N, C, K1 = 16, 1024, 64, 4
NCORES = 8
BPC = B // NCORES  # batches per core
LN_EPS = 1e-5

_NC = None


def _build_module():
    from concourse import bacc, bass, tile, mybir

    f32 = mybir.dt.float32
    bf16 = mybir.dt.bfloat16
    f16 = mybir.dt.float16
    i32 = mybir.dt.int32
    AX = mybir.AxisListType
    OP = mybir.AluOpType
    AF = mybir.ActivationFunctionType

    nc = bacc.Bacc(
        "TRN2",
        target_bir_lowering=False,
        debug=False,
        enable_asserts=False,
    )

    AT_d = nc.dram_tensor(
        "AT16", [BPC, 4, 128, 2, N], f16, kind="ExternalInput"
    ).ap()
    # blobX[p, b*512 + t*64 + c] = X[b, 128t+p, c]; last 2 cols: EYE (1.0
    # at partitions 0/32) for the transpose outer products.
    BX_d = nc.dram_tensor("BX", [128, 2 * 512 + 2], f16, kind="ExternalInput").ap()
    # HBx[p, i, t*64 + c] = h[i, c, 128t+p] (i-major so taps stream in)
    HB01_d = nc.dram_tensor("HB01", [128, 2, 512], bf16, kind="ExternalInput").ap()
    HB23_d = nc.dram_tensor("HB23", [128, 2, 512], bf16, kind="ExternalInput").ap()
    # BF[p, 0:128]  = M2 moments [t, 16], BF[p, 128:160] = HM [t, 4]
    BF_d = nc.dram_tensor("BF", [128, 160], f32, kind="ExternalInput").ap()
    # OUT is partition-major on DRAM (big contiguous DMA descriptors);
    # the host un-permutes to [N, C] after download.
    OUT_d = nc.dram_tensor("OUT", [BPC, 128, 8, C], f32, kind="ExternalOutput").ap()

    with tile.TileContext(nc) as tc:
        with (
            tc.tile_pool(name="big", bufs=2) as big,
            tc.tile_pool(name="aux", bufs=1) as aux,
            tc.tile_pool(name="psum", bufs=2, space="PSUM") as psum,
        ):
            # ---- DMA plan. The SWDGE ring is the only fast one here
            # (~250-300 GB/s); the HWDGE rings crawl (~50-100 GB/s), so
            # they only carry one A chunk per batch (for a little extra
            # aggregate) plus the late-needed blobs. Emission order sets
            # the 8 round-robin completion lanes so collisions only pair
            # with long-finished transfers.
            #   SWDGE: A b0 (j0, j1, j23, j45), HB01, A b1c0-2, OUT halves
            #   ACT:   blobX, A b0j67, A b1c3, HB23, BF
            # b0's first 512KB is split in two so the PE starts ~1us sooner.
            BX_sb = aux.tile([128, 2 * 512 + 2], f16, tag="BX")
            nc.scalar.dma_start(BX_sb, BX_d)

            A0_tiles = [
                aux.tile([128, 1, N], f16, tag="A0j0", name="A0j0"),
                aux.tile([128, 1, N], f16, tag="A0j1", name="A0j1"),
                aux.tile([128, 2, N], f16, tag="A0c1", name="A0c1"),
                aux.tile([128, 2, N], f16, tag="A0c2", name="A0c2"),
                aux.tile([128, 2, N], f16, tag="A0c3", name="A0c3"),
            ]
            A1_tiles = [
                aux.tile([128, 2, N], f16, tag=f"A1c{jp}", name=f"A1c{jp}")
                for jp in range(4)
            ]
            HB_sbs = [
                aux.tile([128, 2, 512], bf16, tag="HB01", name="HB01_sb"),
                aux.tile([128, 2, 512], bf16, tag="HB23", name="HB23_sb"),
            ]
            BF_sb = aux.tile([128, 160], f32, tag="BF")

            nc.gpsimd.dma_start(A0_tiles[0], AT_d[0, 0][:, 0:1])
            nc.gpsimd.dma_start(A0_tiles[1], AT_d[0, 0][:, 1:2])
            nc.gpsimd.dma_start(A0_tiles[2], AT_d[0, 1])
            nc.gpsimd.dma_start(A0_tiles[3], AT_d[0, 2])
            nc.scalar.dma_start(A0_tiles[4], AT_d[0, 3])
            nc.gpsimd.dma_start(HB_sbs[0], HB01_d)
            for jp in range(3):
                nc.gpsimd.dma_start(A1_tiles[jp], AT_d[1, jp])
            nc.scalar.dma_start(A1_tiles[3], AT_d[1, 3])
            nc.scalar.dma_start(HB_sbs[1], HB23_d)
            nc.scalar.dma_start(BF_sb, BF_d)

            # (batch, j) -> moving-operand AP for the 512-wide half q
            def a_ap(b, j, q):
                if b == 0:
                    tl, jj = [(0, 0), (1, 0), (2, 0), (2, 1), (3, 0), (3, 1),
                              (4, 0), (4, 1)][j]
                    t = A0_tiles[tl]
                else:
                    t = A1_tiles[j // 2]
                    jj = j % 2
                return t[:, jj, 512 * q : 512 * (q + 1)]

            def Hvi(i):
                return HB_sbs[i // 2][:, i % 2].rearrange("p (t c) -> p t c", t=8)

            M2_v = BF_sb[:, 0:128].rearrange("p (t z) -> p t z", t=8, z=16)
            HM_v = BF_sb[:, 128:160].rearrange("p (t z) -> p t z", t=8, z=K1)

            Xv = BX_sb[:, 0:1024].rearrange("p (b t c) -> p b t c", b=BPC, t=8)
            EYE_v = BX_sb[0:33, 1024:1025]

            zero_sb = aux.tile([128, 1], f32, tag="zero")
            nc.vector.memset(zero_sb, 0.0)
            zerob_sb = aux.tile([128, 1], bf16, tag="zerob")
            nc.vector.memset(zerob_sb, 0.0)
            magic = aux.tile([128, 8], i32, tag="magic")
            nc.vector.memset(magic, 0x5F3759DF)
            # Preload the Tanh ACT table while DMAs run.
            warm = aux.tile([128, 1], f32, tag="warm")
            nc.scalar.activation(warm, zero_sb, AF.Tanh, bias=zero_sb)

            # colmm: fp16 chain values (P_i/32^i), padded to 2 elems per
            # entry so each [128,1] stationary slice is 4B-aligned.
            # cole[p, t, i] = P_i[128t+p] in bf16 (un-scaled).
            coles = []
            colmms = []
            for b in range(BPC):
                cole = big.tile([128, 8, K1], bf16, tag=f"cole{b}", name=f"cole{b}")
                colmm = big.tile(
                    [128, 8, K1, 2], f16, tag=f"colmm{b}", name=f"colmm{b}"
                )
                with nc.allow_low_precision(reason="Xs cast to 16-bit for matmul"):
                    nc.vector.tensor_reduce(cole[:, :, 0], Xv[:, b], AX.X, OP.add)
                    nc.vector.tensor_reduce(colmm[:, :, 0, 0], Xv[:, b], AX.X, OP.add)
                coles.append(cole)
                colmms.append(colmm)

            # Y accumulators (bf16) built up step by step on DVE.
            Yaccs = [
                big.tile([128, 8, C], bf16, tag=f"Yacc{b}", name=f"Yacc{b}")
                for b in range(BPC)
            ]
            Ytmps = [
                big.tile([128, 8, C], bf16, tag=f"Ytmp{b}", name=f"Ytmp{b}")
                for b in range(BPC)
            ]

            def hterm(dst, b, i, eng):
                colb = (
                    coles[b][:, :, i : i + 1].broadcast_to([128, 8, C])
                )
                eng.tensor_tensor(dst, Hvi(i), colb, OP.mult)

            def acc(i, b):
                # matmul out / stationary base partition must be 0, 32, or
                # 64 -> the two 512-wide halves go to partitions 0/32.
                colmm = colmms[b]
                pr = psum.tile([33, 512], f32, tag=f"pr{b}", name=f"pr{b}")
                for j in range(8):
                    for q in range(2):
                        nc.tensor.matmul(
                            pr[32 * q : 32 * q + 1, :],
                            colmm[:, j, i - 1, 0:1],
                            a_ap(b, j, q),
                            start=(j == 0),
                            stop=(j == 7),
                        )
                return pr

            def tra(i, b, pr):
                # PSUM row -> col layout: n = 512q + 128u + p -> t = 4q + u.
                # Rows cast to fp16 on ACT, then fp16 K=1 outer-product
                # transposes; cole un-scales by 32^i.
                cole = coles[b]
                colmm = colmms[b]
                s2 = big.tile([33, 512], f16, tag=f"s2{b}", name=f"s2{b}")
                for q in range(2):
                    nc.scalar.copy(
                        s2[32 * q : 32 * q + 1, :], pr[32 * q : 32 * q + 1, :]
                    )
                for q in range(2):
                    pt = psum.tile([128, 4, 2], f16, tag=f"pt{b}", name=f"pt{b}")
                    for u in range(4):
                        nc.tensor.matmul(
                            pt[:, u, 0:1],
                            s2[32 * q : 32 * q + 1, 128 * u : 128 * (u + 1)],
                            EYE_v[32 * q : 32 * q + 1, :],
                            is_transpose=True,
                            start=(u == 0),
                            stop=(u == 3),
                        )
                    if i < K1 - 1:
                        nc.scalar.copy(colmm[:, 4 * q : 4 * (q + 1), i, 0], pt[:, :, 0])
                    nc.scalar.activation(
                        cole[:, 4 * q : 4 * (q + 1), i],
                        pt[:, :, 0],
                        AF.Copy,
                        scale=float(32.0**i),
                    )

            def ywin(b, i):
                # After tra(i, b): fold tap i into Yacc (DVE, overlaps the
                # next PE accumulation). Tap 0 pairs with tap 1.
                if i == 1:
                    hterm(Yaccs[b], b, 0, nc.vector)
                    hterm(Ytmps[b], b, 1, nc.vector)
                    nc.vector.tensor_tensor(
                        Yaccs[b], Yaccs[b], Ytmps[b], OP.add
                    )
                else:
                    hterm(Ytmps[b], b, i, nc.vector)
                    nc.vector.tensor_tensor(
                        Yaccs[b], Yaccs[b], Ytmps[b], OP.add
                    )

            def stats(b):
                # LN stats from col + host moments (fp32), right after the
                # last transpose; rstd via Quake + 1 Newton iteration.
                col = coles[b]
                cc = big.tile([128, 8, K1, K1], f32, tag=f"cc{b}")
                nc.vector.tensor_tensor(
                    cc,
                    col.unsqueeze(3).broadcast_to([128, 8, K1, K1]),
                    col.unsqueeze(2).broadcast_to([128, 8, K1, K1]),
                    OP.mult,
                )
                m2t = big.tile([128, 8, K1 * K1], f32, tag=f"m2t{b}")
                nc.vector.tensor_tensor(
                    m2t, cc.rearrange("p t i j -> p t (i j)"), M2_v, OP.mult
                )
                ey2 = big.tile([128, 8], f32, tag=f"ey2{b}")
                nc.vector.tensor_reduce(ey2, m2t, AX.X, OP.add)

                mm4 = big.tile([128, 8, K1], f32, tag=f"mm4{b}")
                nc.vector.tensor_tensor(mm4, col, HM_v, OP.mult)
                mu = big.tile([128, 8], f32, tag=f"mu{b}")
                nc.vector.tensor_reduce(mu, mm4, AX.X, OP.add)
                mu2 = big.tile([128, 8], f32, tag=f"mu2{b}")
                nc.vector.tensor_tensor(mu2, mu, mu, OP.mult)

                veps = big.tile([128, 8], f32, tag=f"veps{b}")
                nc.vector.tensor_tensor(veps, ey2, mu2, OP.subtract)

                rstd = big.tile([128, 8], f32, tag=f"rstd{b}")
                nc.vector.tensor_scalar(
                    rstd.bitcast(i32),
                    veps.bitcast(i32),
                    1,
                    None,
                    OP.logical_shift_right,
                )
                nc.vector.tensor_tensor(
                    rstd.bitcast(i32), magic, rstd.bitcast(i32), OP.subtract
                )
                tq = big.tile([128, 8], f32, tag=f"tq{b}")
                nc.vector.tensor_tensor(tq, rstd, rstd, OP.mult)
                # tq = (-0.5 * tq) * veps ; rstd = (1.5 + tq) * rstd
                nc.vector.scalar_tensor_tensor(tq, tq, -0.5, veps, OP.mult, OP.mult)
                nc.vector.scalar_tensor_tensor(
                    rstd, tq, 1.5, rstd, OP.add, OP.mult
                )

                mur = big.tile([128, 8], f32, tag=f"mur{b}")
                nc.vector.tensor_tensor(mur, mu, rstd, OP.mult)
                rstdh = big.tile([128, 8], bf16, tag=f"rstdh{b}")
                nc.vector.tensor_copy(rstdh, rstd)
                murh = big.tile([128, 8], bf16, tag=f"murh{b}")
                nc.vector.tensor_copy(murh, mur)
                return rstdh, murh

            def finish(b, rstdh, murh):
                # i=3 Y term on GPSIMD (parallel with the DVE stats that
                # just ran), then Yn = Y*rstd - mu*rstd, tanh per half,
                # OUT halves on the idle SP ring.
                hterm(Ytmps[b], b, 3, nc.gpsimd)
                nc.gpsimd.tensor_tensor(Yaccs[b], Yaccs[b], Ytmps[b], OP.add)

                Yn = big.tile([128, 8, C], bf16, tag=f"Yn{b}")
                rstdb = rstdh.unsqueeze(2).broadcast_to([128, 8, C])
                nc.vector.tensor_tensor(Yn, Yaccs[b], rstdb, OP.mult)
                murb = murh.unsqueeze(2).broadcast_to([128, 8, C])
                nc.vector.tensor_tensor(Yn, Yn, murb, OP.subtract)

                OUT_sb = big.tile([128, 8, C], f32, tag=f"OUTS{b}")
                for half in range(2):
                    sl = slice(4 * half, 4 * half + 4)
                    nc.scalar.activation(
                        OUT_sb[:, sl], Yn[:, sl], AF.Tanh, bias=zerob_sb
                    )
                    nc.gpsimd.dma_start(OUT_d[b][:, sl], OUT_sb[:, sl])

            # Software pipeline: transposes trail the next accumulation by
            # one step; Y taps fold in on DVE right after each transpose.
            pr = acc(1, 0)
            pr1 = acc(1, 1)
            tra(1, 0, pr)
            pr = acc(2, 0)
            tra(1, 1, pr1)
            ywin(0, 1)
            pr1 = acc(2, 1)
            tra(2, 0, pr)
            ywin(1, 1)
            pr = acc(3, 0)
            tra(2, 1, pr1)
            ywin(0, 2)
            tra(3, 0, pr)
            ywin(1, 2)
            r0 = stats(0)
            pr1 = acc(3, 1)
            tra(3, 1, pr1)
            finish(0, *r0)
            r1 = stats(1)
            finish(1, *r1)

    nc.compile()
    return nc


def _get_module():
    global _NC
    if _NC is None:
        _NC = _build_module()
    return _NC


def _make_in_maps(A, X, h):
    import ml_dtypes

    bf16 = ml_dtypes.bfloat16
    # AT16[b, j, p, n] = A[b, n, 128j + p] / 32  (A^T chunked by 128 m-rows;
    # the 1/32 keeps every P_i' = P_i/32^i in fp16 range, undone on-device
    # by the 32^i scale on the cole copies)
    AT = A.transpose(0, 2, 1).reshape(B, 4, 2, 128, N).transpose(0, 1, 3, 2, 4)
    AT16 = (AT / np.float32(32.0)).astype(np.float16)

    # blobX[p, b*512 + t*64 + c] = X[b, 128t+p, c], plus the EYE column.
    X16 = X.astype(np.float16).reshape(B, 8, 128, C)

    # HBx[p, i, t*64+c] = h[i, c, 128t+p]  (i-major, split in two)
    Hh = h.transpose(0, 2, 1).reshape(K1, 8, 128, C).transpose(2, 0, 1, 3)
    HBa = np.ascontiguousarray(Hh.reshape(128, K1, 512)).astype(bf16)
    HB01 = np.ascontiguousarray(HBa[:, 0:2])
    HB23 = np.ascontiguousarray(HBa[:, 2:4])

    # Host LN moments: HM[n, i] = mean_c h[i,c,n]; M2[n, i*4+j] = mean_c h_i h_j
    hf = h.astype(np.float64)
    HMF = hf.mean(axis=1).T.astype(np.float32)  # [N, K1]
    M2F = (np.einsum("icn,jcn->nij", hf, hf) / C).reshape(N, K1 * K1)
    M2F = M2F.astype(np.float32)
    # BF[p, 0:128] = M2[t, z]; BF[p, 128:160] = HM[t, z]
    BF = np.concatenate(
        [
            M2F.reshape(8, 128, 16).transpose(1, 0, 2).reshape(128, 128),
            HMF.reshape(8, 128, K1).transpose(1, 0, 2).reshape(128, 32),
        ],
        axis=1,
    )
    BF = np.ascontiguousarray(BF, dtype=np.float32)

    in_maps = []
    for core in range(NCORES):
        sl = slice(BPC * core, BPC * (core + 1))
        Xc = X16[sl]  # [BPC, 8, 128, C]
        BX = np.zeros((128, 2 * 512 + 2), dtype=np.float16)
        BX[:, 0:1024] = Xc.transpose(2, 0, 1, 3).reshape(128, BPC * 512)
        BX[0, 1024] = 1.0
        BX[32, 1024] = 1.0
        in_maps.append(
            {
                "AT16": np.ascontiguousarray(AT16[sl]),
                "BX": BX,
                "HB01": HB01,
                "HB23": HB23,
                "BF": BF,
            }
        )
    return in_maps


def _numpy_fallback(A, X, h, ln_gamma, ln_beta):
    Xs = X.sum(-1)
    p = Xs
    powers = [Xs]
    for _ in range(K1 - 1):
        p = np.einsum("bnm,bm->bn", A, p)
        powers.append(p)
    P = np.stack(powers)
    Y = np.einsum("icn,ibn->bnc", h, P)
    mu = Y.mean(axis=-1, keepdims=True)
    var = Y.var(axis=-1, keepdims=True)
    Yn = (Y - mu) / np.sqrt(var + LN_EPS) * ln_gamma + ln_beta
    return np.tanh(Yn).astype(np.float32)


def _run(A, X, h, ln_gamma, ln_beta, trace=False):
    A = np.ascontiguousarray(np.asarray(A, dtype=np.float32))
    X = np.ascontiguousarray(np.asarray(X, dtype=np.float32))
    h = np.ascontiguousarray(np.asarray(h, dtype=np.float32))
    g = np.asarray(ln_gamma, dtype=np.float32)
    be = np.asarray(ln_beta, dtype=np.float32)

    if not (np.all(g == 1.0) and np.all(be == 0.0)):
        # device kernel folds the (identity) affine away; anything else is
        # handled on host
        return _numpy_fallback(A, X, h, g, be), None

    from concourse import bass_utils

    nc = _get_module()
    res = bass_utils.run_bass_kernel_spmd(
        nc, _make_in_maps(A, X, h), core_ids=list(range(NCORES)), trace=trace
    )
    # un-permute the partition-major device layout: n = 128t + p
    out = np.concatenate(
        [
            np.asarray(r["OUT"]).transpose(0, 2, 1, 3).reshape(BPC, N, C)
            for r in res.results
        ],
        axis=0,
    )
    return out.astype(np.float32, copy=False), res.exec_time_ns


def kernel(A, X, h, ln_gamma, ln_beta):
    out, _ = _run(A, X, h, ln_gamma, ln_beta, trace=False)
    return out


def kernel_profiled(A, X, h, ln_gamma, ln_beta):
    return _run(A, X, h, ln_gamma, ln_beta, trace=True)
